# revision 36
# baseline (speedup 1.0000x reference)
"""CompressedGPT2Attention on 8 TRN2 NeuronCores (~216us, 1.41x vs v1).

Sharding: core c = (batch b = c // 2, head-group g = c % 2) — data parallel on
B=4, tensor parallel over 16 heads (8 per group). Each core computes a partial
output [S, E] bf16 (its head-group's contribution); the host sums the two
partials per batch in f32 and adds output_bias.

Design (evolved from the 304.8us f32r v1 via TimelineSim-driven iteration):
  * Projections (q/k/v) run as fp8e4m3 DoubleRow matmuls: host supplies each
    operand as (value, residual) fp8 pairs in a chunk-pair plane layout
    [128, c=4, plane=2, *], so one matmul contracts 256 e-dims at 0.5
    cycles/col — 4x f32r throughput; 3 terms (w*x + w*xr + wr*x) restore
    ~bf16 accuracy. Weights/biases are pre-scaled 64x to keep fp8 normal;
    the 64x on v cancels in softmax normalization, the 64^2 on q*k folds
    into the exp scale.
  * Attention (q*k^T scores, exp, probs*v) is bf16 (1.0 cyc/col at any span;
    fp8 would not survive the 2e-2 gate — prob/v element errors do not
    average out in the output). The denominator comes free as a 33rd
    ones-column of v_aug. Causal masking: exp over [lo,1024) then one DVE
    multiply of the 128-col diagonal block by an upper-triangular tile.
  * The attention phase is ACT(exp)-bound (~159us ACT vs ~130us attention
    PE), so all remaining PE work — q/k projections m=1..3, v chunks 12..15,
    and the row 0..1023 output projection — is queued as ~0.7us "foreign"
    units popped into the per-head instruction stream at tuned points, with
    unit order respecting data deps (head 2m needs q/k pair m; ib2=1 needs
    q cols 1024:2048 up front but k chunk jc only by step jc).
  * psum: scores [128,1024]x2 (4 banks) + attn accum [33,1024] (2 banks) +
    foreign/outproj [128,512]x2 (2 banks) = all 8 banks.
  * 24 big input DMAs ordered by first use (host pre-interleaves layouts);
    outputs are stored bf16, one [128, E] DMA per 128 rows.
"""

import numpy as np
import ml_dtypes
from contextlib import ExitStack

import concourse.bass as bass
import concourse.bacc as bacc
import concourse.tile as tile
import concourse.mybir as mybir
from concourse.bass_utils import run_bass_kernel_spmd

F32 = mybir.dt.float32
BF16 = mybir.dt.bfloat16
FP8 = mybir.dt.float8e4
DR = mybir.MatmulPerfMode.DoubleRow
AF = mybir.ActivationFunctionType

B, S, E = 4, 2048, 1024
H, HD, R = 16, 64, 32
HG = 8                # heads per core
N_CORES = 8
WS = 64.0             # host pre-scale on w/bias (keeps fp8 out of subnormals);
                      # q,k,v all carry a 64x factor on device. v's factor
                      # cancels in softmax normalization (numerator and
                      # denominator both 64x); q*k's 64^2 folds into exp scale.
SCALE = 1.0 / 8.0 / (WS * WS)   # 1/sqrt(HD) / 64^2
VC = HG * 33          # v_aug cols: per head 32 v-cols + ones col

_PROGRAM_CACHE = {}
MM_LABELS = []  # build-order labels for every nc.tensor.matmul (for analyze.py)


def _build_program():
    nc = bacc.Bacc("TRN2", target_bir_lowering=False, debug=False,
                   num_devices=N_CORES)

    # fp8 inputs come as (value, residual) pairs in chunk-pair plane layout
    # [128, c=4, plane=2, *]: plane p of pair c holds e-chunk 2c+p, so a
    # DoubleRow matmul contracts 256 e-dims per instruction at 0.5 cyc/col.
    hs_d = nc.dram_tensor("hs", [128, 4, 2, S], FP8, kind="ExternalInput").ap()
    hsr_d = nc.dram_tensor("hsr", [128, 4, 2, S], FP8, kind="ExternalInput").ap()
    wq_d = nc.dram_tensor("wq", [128, 4, 2, 512], FP8, kind="ExternalInput").ap()
    wqr_d = nc.dram_tensor("wqr", [128, 4, 2, 512], FP8, kind="ExternalInput").ap()
    wk_d = nc.dram_tensor("wk", [128, 4, 2, 512], FP8, kind="ExternalInput").ap()
    wkr_d = nc.dram_tensor("wkr", [128, 4, 2, 512], FP8, kind="ExternalInput").ap()
    bqt_d = nc.dram_tensor("bqt", [128, 4], F32, kind="ExternalInput").ap()
    bkt_d = nc.dram_tensor("bkt", [128, 4], F32, kind="ExternalInput").ap()
    wv_d = nc.dram_tensor("wv", [128, 4, 2, VC], FP8, kind="ExternalInput").ap()
    wvr_d = nc.dram_tensor("wvr", [128, 4, 2, VC], FP8, kind="ExternalInput").ap()
    bv_d = nc.dram_tensor("bv", [1, VC], F32, kind="ExternalInput").ap()
    wout_d = nc.dram_tensor("wout", [128, 2, E], BF16, kind="ExternalInput").ap()
    tri_d = nc.dram_tensor("tri", [128, 128], BF16, kind="ExternalInput").ap()
    out_d = nc.dram_tensor("out", [S, E], BF16, kind="ExternalOutput").ap()

    with tile.TileContext(nc) as tc, ExitStack() as ctx:
        persist = ctx.enter_context(tc.tile_pool(name="persist", bufs=1))

        hs_sb = persist.tile([128, 4, 2, S], FP8, name="hs", tag="hs")
        hsr_sb = persist.tile([128, 4, 2, S], FP8, name="hsr", tag="hsr")
        wq_sb = persist.tile([128, 4, 2, 512], FP8, name="wq", tag="wq")
        wqr_sb = persist.tile([128, 4, 2, 512], FP8, name="wqr", tag="wqr")
        wk_sb = persist.tile([128, 4, 2, 512], FP8, name="wk", tag="wk")
        wkr_sb = persist.tile([128, 4, 2, 512], FP8, name="wkr", tag="wkr")
        wv_sb = persist.tile([128, 4, 2, VC], FP8, name="wv", tag="wv")
        wvr_sb = persist.tile([128, 4, 2, VC], FP8, name="wvr", tag="wvr")
        wout_sb = persist.tile([128, 2, E], BF16, name="wo", tag="wo")
        tri_sb = persist.tile([128, 128], BF16, name="tri", tag="tri")
        bqt_sb = persist.tile([128, 4], F32, name="bqt", tag="bqt")
        bkt_sb = persist.tile([128, 4], F32, name="bkt", tag="bkt")
        bv_sb = persist.tile([1, VC], F32, name="bv", tag="bv")
        bv_bc = persist.tile([128, VC], F32, name="bvbc", tag="bvbc")

        q_sb = [persist.tile([128, S], BF16, name=f"q{m}", tag=f"q{m}")
                for m in range(4)]
        k_sb = [persist.tile([128, S], BF16, name=f"k{m}", tag=f"k{m}")
                for m in range(4)]
        v_sb = [persist.tile([128, VC], BF16, name=f"v{sc}", tag=f"v{sc}")
                for sc in range(16)]
        attn_sb = [persist.tile([128, S], BF16, name=f"attn{t}", tag=f"attn{t}")
                   for t in range(2)]

        # ---- input DMAs, ordered by first use (12 loads) ----
        dma = nc.sync.dma_start
        dma(out=hs_sb[:, :, :, 0:256], in_=hs_d[:, :, :, 0:256])
        dma(out=wv_sb[:, 0:2, :, :], in_=wv_d[:, 0:2, :, :])
        dma(out=wv_sb[:, 2:4, :, :], in_=wv_d[:, 2:4, :, :])
        dma(out=hsr_sb[:, :, :, 0:256], in_=hsr_d[:, :, :, 0:256])
        dma(out=wvr_sb, in_=wvr_d)
        dma(out=bv_sb, in_=bv_d)
        dma(out=hs_sb[:, :, :, 256:512], in_=hs_d[:, :, :, 256:512])
        dma(out=hsr_sb[:, :, :, 256:512], in_=hsr_d[:, :, :, 256:512])
        dma(out=hs_sb[:, :, :, 512:1024], in_=hs_d[:, :, :, 512:1024])
        dma(out=hsr_sb[:, :, :, 512:1024], in_=hsr_d[:, :, :, 512:1024])
        dma(out=wq_sb, in_=wq_d)
        dma(out=wqr_sb, in_=wqr_d)
        dma(out=bqt_sb, in_=bqt_d)
        dma(out=bkt_sb, in_=bkt_d)
        dma(out=hs_sb[:, :, :, 1024:1536], in_=hs_d[:, :, :, 1024:1536])
        dma(out=hsr_sb[:, :, :, 1024:1536], in_=hsr_d[:, :, :, 1024:1536])
        dma(out=wk_sb, in_=wk_d)
        dma(out=wkr_sb, in_=wkr_d)
        dma(out=hs_sb[:, :, :, 1536:2048], in_=hs_d[:, :, :, 1536:2048])
        dma(out=hsr_sb[:, :, :, 1536:2048], in_=hsr_d[:, :, :, 1536:2048])
        dma(out=tri_sb, in_=tri_d)
        dma(out=wout_sb, in_=wout_d)

        nc.gpsimd.partition_broadcast(bv_bc, bv_sb)

        # ---- helpers ----
        def v_group(pool, sc, bufs):
            # v_aug (64x) = sum over chunk pairs of (hs8+hsr8)^T (wv8+wvr8),
            # dropping the tiny hsr*wvr term; 12 DoubleRow matmuls.
            ps = pool.tile([128, VC], F32, name="vp", tag="vp", bufs=bufs)
            ssl = slice(sc * 128, (sc + 1) * 128)
            terms = [(hs_sb, wv_sb), (hs_sb, wvr_sb), (hsr_sb, wv_sb)]
            n = 0
            for ht, wt in terms:
                for c in range(4):
                    nc.tensor.matmul(ps, ht[:, c, :, ssl], wt[:, c, :, :],
                                     start=(n == 0), stop=(n == 11),
                                     perf_mode=DR)
                    n += 1
            nc.vector.tensor_add(out=v_sb[sc], in0=ps, in1=bv_bc)

        def qk_group(pool, tag, bufs, which, m, nb, copy_engine):
            w_sb, wr_sb, bias_sb, dst = (
                (wq_sb, wqr_sb, bqt_sb, q_sb) if which == "q"
                else (wk_sb, wkr_sb, bkt_sb, k_sb))
            sl = slice(nb * 512, nb * 512 + 512)
            msl = slice(m * 128, (m + 1) * 128)
            ps = pool.tile([128, 512], F32, name="pj", tag=tag, bufs=bufs)
            n = 0
            for ht, wt in ((hs_sb, w_sb), (hs_sb, wr_sb), (hsr_sb, w_sb)):
                for c in range(4):
                    nc.tensor.matmul(ps, wt[:, c, :, msl], ht[:, c, :, sl],
                                     start=(n == 0), stop=(n == 11),
                                     perf_mode=DR)
                    n += 1
            if copy_engine == "act":
                nc.scalar.activation(out=dst[m][:, sl], in_=ps, func=AF.Identity,
                                     bias=bias_sb[:, m:m + 1], scale=1.0)
            else:
                nc.vector.tensor_scalar_add(out=dst[m][:, sl], in0=ps,
                                            scalar1=bias_sb[:, m:m + 1])

        def outproj_pair(pool, tag, bufs, ob_pool, it, copy_engines):
            # both 512-col halves of rows it*128..+128, one merged store
            ot = ob_pool.tile([128, E], BF16, name="ot", tag="ot")
            for eb in range(2):
                sl = slice(eb * 512, eb * 512 + 512)
                ps = pool.tile([128, 512], F32, name="op", tag=tag, bufs=bufs)
                for t in range(2):
                    nc.tensor.matmul(ps, attn_sb[t][:, it * 128:(it + 1) * 128],
                                     wout_sb[:, t, sl],
                                     start=(t == 0), stop=(t == 1))
                if copy_engines[eb] == "act":
                    nc.scalar.activation(out=ot[:, sl], in_=ps, func=AF.Copy,
                                         bias=0.0, scale=1.0)
                else:
                    nc.vector.tensor_copy(out=ot[:, sl], in_=ps)
            nc.sync.dma_start(out=out_d[it * 128:(it + 1) * 128, :], in_=ot)

        # ---- phase 1a: v projection + q/k m=0 cols 0..1024 ----
        with ExitStack() as p1:
            pp0 = p1.enter_context(tc.tile_pool(name="pp0", bufs=1, space="PSUM"))
            for sc in range(12):
                v_group(pp0, sc, bufs=4)
            qk_group(pp0, "pj", 2, "q", 0, 0, "act")
            qk_group(pp0, "pj", 2, "q", 0, 1, "dve")
            qk_group(pp0, "pj", 2, "k", 0, 0, "act")
            qk_group(pp0, "pj", 2, "k", 0, 1, "dve")

        # ---- attention (+ interleaved foreign PE work) ----
        with ExitStack() as actx:
            sp_pool = actx.enter_context(tc.tile_pool(name="spp", bufs=1, space="PSUM"))
            at_pool = actx.enter_context(tc.tile_pool(name="atp", bufs=1, space="PSUM"))
            fp_pool = actx.enter_context(tc.tile_pool(name="fpp", bufs=1, space="PSUM"))
            et_pool = actx.enter_context(tc.tile_pool(name="etp", bufs=4))
            nrm_pool = actx.enter_context(tc.tile_pool(name="nrm", bufs=2))
            ob_pool = actx.enter_context(tc.tile_pool(name="obp", bufs=4))

            # foreign PE work queues; order respects data deps:
            # ib2=0 head h=2m needs q/k m nb0/nb1 emitted by end of head 2m-1;
            # ib2=1 head h uses q cols 1024:2048 (nb2/3) and k chunks jc>=8.
            # qk groups are split into two 4-chunk half-units (same psum tile,
            # accumulation group spans both) so foreign work comes in ~0.85us
            # units that slot between attention chunks at fine grain.
            def qk_units(which, m, nb):
                w_sb, wr_sb, bias_sb, dst = (
                    (wq_sb, wqr_sb, bqt_sb, q_sb) if which == "q"
                    else (wk_sb, wkr_sb, bkt_sb, k_sb))
                sl = slice(nb * 512, nb * 512 + 512)
                msl = slice(m * 128, (m + 1) * 128)
                parts = [(ht, wt, c)
                         for ht, wt in ((hs_sb, w_sb), (hs_sb, wr_sb),
                                        (hsr_sb, w_sb))
                         for c in range(4)]
                state = {}

                def half(lohalf):
                    def emit():
                        if "ps" not in state:
                            state["ps"] = fp_pool.tile([128, 512], F32,
                                                       name="pj", tag="fp",
                                                       bufs=2)
                        ps = state["ps"]
                        rng = range(0, 6) if lohalf else range(6, 12)
                        for n in rng:
                            ht, wt, c = parts[n]
                            nc.tensor.matmul(ps, wt[:, c, :, msl],
                                             ht[:, c, :, sl],
                                             start=(n == 0), stop=(n == 11),
                                             perf_mode=DR)
                        if not lohalf:
                            nc.vector.tensor_scalar_add(
                                out=dst[m][:, sl], in0=ps,
                                scalar1=bias_sb[:, m:m + 1])
                    return emit
                return [half(True), half(False)]

            def op_unit(it):
                def emit():
                    outproj_pair(fp_pool, "fp", 2, ob_pool, it, ("dve", "dve"))
                return emit

            def vp_unit(sc):
                # v projection group as foreign work; borrows an fp-pool slot
                # (tile is [128,512] f32, v psum uses the first 264 cols)
                def emit():
                    ps = fp_pool.tile([128, 512], F32, name="pj", tag="fp",
                                      bufs=2)
                    ssl = slice(sc * 128, (sc + 1) * 128)
                    n = 0
                    for ht, wt in ((hs_sb, wv_sb), (hs_sb, wvr_sb),
                                   (hsr_sb, wv_sb)):
                        for c in range(4):
                            nc.tensor.matmul(ps[:, 0:VC], ht[:, c, :, ssl],
                                             wt[:, c, :, :],
                                             start=(n == 0), stop=(n == 11),
                                             perf_mode=DR)
                            n += 1
                    nc.vector.tensor_add(out=v_sb[sc], in0=ps[:, 0:VC],
                                         in1=bv_bc)
                return emit

            foreignA = []
            for which, m, nb in (("q", 1, 0), ("q", 1, 1), ("k", 1, 0),
                                 ("k", 1, 1), ("q", 2, 0), ("q", 2, 1),
                                 ("k", 2, 0), ("k", 2, 1), ("q", 3, 0),
                                 ("q", 3, 1), ("k", 3, 0), ("k", 3, 1),
                                 ("q", 0, 2), ("q", 0, 3), ("k", 0, 2),
                                 ("k", 0, 3)):
                foreignA += qk_units(which, m, nb)
            foreignB = []
            for which, m, nb in (("q", 1, 2), ("q", 1, 3), ("k", 1, 2),
                                 ("k", 1, 3), ("q", 2, 2), ("q", 2, 3),
                                 ("k", 2, 2), ("k", 2, 3), ("q", 3, 2),
                                 ("q", 3, 3), ("k", 3, 2), ("k", 3, 3)):
                foreignB += qk_units(which, m, nb)
            # order check (deps): q m nb2+nb3 complete before head 2m of
            # ib2=1 starts; k m nb2 before that head's jc=8, nb3 before jc=12

            foreignB += [op_unit(it) for it in range(8)]
            foreignB = [vp_unit(sc) for sc in range(12, 16)] + foreignB
            fq = {0: foreignA, 1: foreignB}

            # pop budgets tuned to the per-head ACT deficit: late ib2=1 heads
            # (where proj work has run out) get the outproj units
            def points_for(ib2, h):
                if ib2 == 0:
                    return (2, 4, 6)          # + boundary = 4 units/head
                if h == 0:
                    return (1, 2, 3, 5, 7)    # + boundary = 6 (vp12..15 early)
                if h < 6:
                    return (4, 8, 12)         # + boundary = 4 units/head
                return (3, 6, 9, 12)          # + boundary = 5 units/head

            def pop_foreign(ib2):
                q = fq[ib2]
                if q:
                    q.pop(0)()

            for ib2 in range(2):
                ibase = ib2 * 1024
                jcs = list(range(8 * (ib2 + 1)))
                ilo = {jc: max(jc * 128 - ibase, 0) for jc in jcs}
                bank_jcs = {nb: [jc for jc in jcs if ilo[jc] < nb * 512 + 512]
                            for nb in range(2)}

                for h in range(8):
                    pairm = h // 2
                    dpart = slice((h % 2) * 64, (h % 2) * 64 + 64)
                    at_ps = at_pool.tile([33, 1024], F32, name="at", tag="at",
                                         bufs=1)
                    ets = {}

                    def emit_qk_exp(jc):
                        lo = ilo[jc]
                        sp = sp_pool.tile([128, 1024], F32, name="sp", tag="sp",
                                          bufs=2)
                        for nb in range(2):
                            a = max(lo, nb * 512)
                            bb = nb * 512 + 512
                            if a >= bb:
                                continue
                            nc.tensor.matmul(
                                sp[:, a:bb],
                                k_sb[pairm][dpart, jc * 128:(jc + 1) * 128],
                                q_sb[pairm][dpart, ibase + a:ibase + bb],
                                start=True, stop=True)
                        et = et_pool.tile([128, 1024], BF16, name="et", tag="et",
                                          bufs=4)
                        nc.scalar.activation(out=et[:, lo:1024], in_=sp[:, lo:1024],
                                             func=AF.Exp, scale=SCALE)
                        if jc >= 8 * ib2:  # diagonal block: causal mask
                            nc.vector.tensor_mul(out=et[:, lo:lo + 128],
                                                 in0=et[:, lo:lo + 128],
                                                 in1=tri_sb)
                        ets[jc] = et

                    def emit_pv(jc):
                        lo = ilo[jc]
                        et = ets.pop(jc)
                        for nb in range(2):
                            a = max(lo, nb * 512)
                            bb = nb * 512 + 512
                            if a >= bb:
                                continue
                            nc.tensor.matmul(
                                at_ps[:, a:bb],
                                v_sb[jc][:, h * 33:(h + 1) * 33],
                                et[:, a:bb],
                                start=(jc == bank_jcs[nb][0]),
                                stop=(jc == bank_jcs[nb][-1]))

                    pts = points_for(ib2, h)
                    for idx, jc in enumerate(jcs):
                        emit_qk_exp(jc)
                        if idx >= 1:
                            emit_pv(jcs[idx - 1])
                        if idx in pts:
                            pop_foreign(ib2)
                    emit_pv(jcs[-1])
                    pop_foreign(ib2)

                    # normalize rows 0..31 by 1/row32 (half-copies free psum
                    # banks early for the next head's PV start). The very last
                    # head is normalized in column halves so the tail outproj
                    # unblocks ~1us earlier.
                    t, roff = h // 4, (h % 4) * 32
                    araw = nrm_pool.tile([33, 1024], BF16, name="araw", tag="araw")
                    rec = nrm_pool.tile([1, 1024], BF16, name="rec", tag="rec")
                    rec_bc = nrm_pool.tile([32, 1024], BF16, name="recbc",
                                           tag="recbc")
                    halves = ((0, 1024),) if not (ib2 == 1 and h == 7) \
                        else ((0, 512), (512, 1024))
                    for (ca, cb) in halves:
                        nc.vector.tensor_copy(out=araw[:, ca:ca + (cb - ca) // 2],
                                              in_=at_ps[:, ca:ca + (cb - ca) // 2])
                        nc.vector.tensor_copy(out=araw[:, ca + (cb - ca) // 2:cb],
                                              in_=at_ps[:, ca + (cb - ca) // 2:cb])
                        with nc.allow_low_precision(reason="bf16 softmax denom"):
                            nc.vector.reciprocal(out=rec[:, ca:cb],
                                                 in_=araw[32:33, ca:cb])
                        nc.gpsimd.partition_broadcast(rec_bc[:, ca:cb],
                                                      rec[:, ca:cb])
                        nc.vector.tensor_mul(
                            out=attn_sb[t][roff:roff + 32, ibase + ca:ibase + cb],
                            in0=araw[0:32, ca:cb], in1=rec_bc[:, ca:cb])

            while fq[1]:
                pop_foreign(1)


        # ---- tail: outproj rows 1024..2047 ----
        with ExitStack() as tctx:
            tp = tctx.enter_context(tc.tile_pool(name="tp", bufs=1, space="PSUM"))
            obt = tctx.enter_context(tc.tile_pool(name="obt", bufs=4))
            for it in range(8, 16):
                outproj_pair(tp, "tp", 6, obt, it, ("act", "dve"))

    nc.compile()
    return nc


def _get_program():
    if "nc" not in _PROGRAM_CACHE:
        _PROGRAM_CACHE["nc"] = _build_program()
    return _PROGRAM_CACHE["nc"]


E4M3 = ml_dtypes.float8_e4m3  # what the stack maps dt.float8e4 to


def _fp8_pair(a):
    """[E, N] f32 -> (value, residual) in chunk-pair plane layout
    [128, 4, 2, N] fp8: plane p of pair c holds rows (2c+p)*128..+128."""
    a8 = a.astype(E4M3)
    r8 = (a - a8.astype(np.float32)).astype(E4M3)
    out = []
    for x in (a8, r8):
        x = x.reshape(4, 2, 128, *a.shape[1:])
        x = np.ascontiguousarray(x.transpose(2, 0, 1, 3))
        out.append(x)
    return out


def kernel(hidden_states, q_weight, q_bias, k_weight, k_bias,
           low_rank_value_weight, low_rank_value_bias,
           low_rank_output_weight, output_bias):
    hidden_states = np.asarray(hidden_states, dtype=np.float32)
    q_weight = np.asarray(q_weight, dtype=np.float32)
    q_bias = np.asarray(q_bias, dtype=np.float32)
    k_weight = np.asarray(k_weight, dtype=np.float32)
    k_bias = np.asarray(k_bias, dtype=np.float32)
    wv_full = np.asarray(low_rank_value_weight, dtype=np.float32)
    bv_full = np.asarray(low_rank_value_bias, dtype=np.float32)
    wout_full = np.asarray(low_rank_output_weight, dtype=np.float32)
    output_bias = np.asarray(output_bias, dtype=np.float32)

    tri = np.triu(np.ones((128, 128), np.float32)).astype(ml_dtypes.bfloat16)
    ws = np.float32(WS)

    in_maps = []
    for c in range(N_CORES):
        b, g = c // 2, c % 2
        hs_t = np.ascontiguousarray(hidden_states[b].T)          # [E, S]
        cols = slice(g * 512, (g + 1) * 512)                     # q/k head cols
        vcols = slice(g * 256, (g + 1) * 256)                    # v head cols
        wv_aug = np.zeros((E, VC), dtype=np.float32)
        bv_aug = np.zeros((1, VC), dtype=np.float32)
        wv_g = wv_full[:, vcols].reshape(E, HG, R)
        bv_g = bv_full[vcols].reshape(HG, R)
        for h in range(HG):
            wv_aug[:, h * 33:h * 33 + 32] = wv_g[:, h, :]
            bv_aug[0, h * 33:h * 33 + 32] = bv_g[h]
            bv_aug[0, h * 33 + 32] = 1.0
        hs8, hsr8 = _fp8_pair(hs_t)
        wq8, wqr8 = _fp8_pair(ws * q_weight[:, cols])
        wk8, wkr8 = _fp8_pair(ws * k_weight[:, cols])
        wv8, wvr8 = _fp8_pair(ws * wv_aug)
        wout_r = np.ascontiguousarray(
            wout_full[vcols, :].reshape(2, 128, E).transpose(1, 0, 2)
        ).astype(ml_dtypes.bfloat16)
        in_maps.append({
            "hs": hs8, "hsr": hsr8,
            "wq": wq8, "wqr": wqr8,
            "wk": wk8, "wkr": wkr8,
            "bqt": np.ascontiguousarray(ws * q_bias[cols].reshape(4, 128).T),
            "bkt": np.ascontiguousarray(ws * k_bias[cols].reshape(4, 128).T),
            "wv": wv8, "wvr": wvr8,
            "bv": ws * bv_aug,
            "wout": wout_r,
            "tri": tri,
        })

    nc = _get_program()
    res = run_bass_kernel_spmd(nc, in_maps, list(range(N_CORES)))
    out = np.empty((B, S, E), dtype=np.float32)
    for b in range(B):
        out[b] = (res.results[2 * b]["out"].astype(np.float32)
                  + res.results[2 * b + 1]["out"].astype(np.float32)
                  + output_bias[None, :])
    return out


# revision 43
# speedup vs baseline: 1.0064x; 1.0064x over previous
"""CompressedGPT2Attention on 8 TRN2 NeuronCores (~214.6us, 1.42x vs v1).

Sharding: core c = (batch b = c // 2, head-group g = c % 2) — data parallel on
B=4, tensor parallel over 16 heads (8 per group). Each core computes a partial
output [S, E] bf16 (its head-group's contribution); the host sums the two
partials per batch in f32 and adds output_bias.

Design (evolved from the 304.8us f32r v1 via TimelineSim-driven iteration):
  * Projections (q/k/v) run as fp8e4m3 DoubleRow matmuls: host supplies each
    operand as (value, residual) fp8 pairs in a chunk-pair plane layout
    [128, c=4, plane=2, *], so one matmul contracts 256 e-dims at 0.5
    cycles/col — 4x f32r throughput; 3 terms (w*x + w*xr + wr*x) restore
    ~bf16 accuracy. Weights/biases are pre-scaled 64x to keep fp8 normal;
    the 64x on v cancels in softmax normalization, the 64^2 on q*k folds
    into the exp scale.
  * Attention (q*k^T scores, exp, probs*v) is bf16 (1.0 cyc/col at any span;
    fp8 would not survive the 2e-2 gate — prob/v element errors do not
    average out in the output). The denominator comes free as a 33rd
    ones-column of v_aug. Causal masking: exp over [lo,1024) then one DVE
    multiply of the 128-col diagonal block by an upper-triangular tile.
  * The attention phase is ACT(exp)-bound (~159us ACT vs ~130us attention
    PE), so all remaining PE work — q/k projections m=1..3, v chunks 12..15,
    and the row 0..1023 output projection — is queued as ~0.7us "foreign"
    units popped into the per-head instruction stream at tuned points, with
    unit order respecting data deps (head 2m needs q/k pair m; ib2=1 needs
    q cols 1024:2048 up front but k chunk jc only by step jc).
  * psum: scores [128,1024]x2 (4 banks) + attn accum [33,1024] (2 banks) +
    foreign/outproj [128,512]x2 (2 banks) = all 8 banks. The attn accumulator
    is normalized per 512-col bank as soon as that bank's last chunk lands
    (bank 0 stops at chunk 11, not 15), freeing psum for the next head early
    and unlocking the tail outproj ~4 chunks sooner.
  * 24 big input DMAs ordered by first use (host pre-interleaves layouts);
    outputs are stored bf16, one [128, E] DMA per 128 rows.
"""

import numpy as np
import ml_dtypes
from contextlib import ExitStack

import concourse.bass as bass
import concourse.bacc as bacc
import concourse.tile as tile
import concourse.mybir as mybir
from concourse.bass_utils import run_bass_kernel_spmd

F32 = mybir.dt.float32
BF16 = mybir.dt.bfloat16
FP8 = mybir.dt.float8e4
DR = mybir.MatmulPerfMode.DoubleRow
AF = mybir.ActivationFunctionType

B, S, E = 4, 2048, 1024
H, HD, R = 16, 64, 32
HG = 8                # heads per core
N_CORES = 8
WS = 64.0             # host pre-scale on w/bias (keeps fp8 out of subnormals);
                      # q,k,v all carry a 64x factor on device. v's factor
                      # cancels in softmax normalization (numerator and
                      # denominator both 64x); q*k's 64^2 folds into exp scale.
SCALE = 1.0 / 8.0 / (WS * WS)   # 1/sqrt(HD) / 64^2
VC = HG * 33          # v_aug cols: per head 32 v-cols + ones col

_PROGRAM_CACHE = {}
MM_LABELS = []  # build-order labels for every nc.tensor.matmul (for analyze.py)


def _build_program():
    nc = bacc.Bacc("TRN2", target_bir_lowering=False, debug=False,
                   num_devices=N_CORES)

    # fp8 inputs come as (value, residual) pairs in chunk-pair plane layout
    # [128, c=4, plane=2, *]: plane p of pair c holds e-chunk 2c+p, so a
    # DoubleRow matmul contracts 256 e-dims per instruction at 0.5 cyc/col.
    hs_d = nc.dram_tensor("hs", [128, 4, 2, S], FP8, kind="ExternalInput").ap()
    hsr_d = nc.dram_tensor("hsr", [128, 4, 2, S], FP8, kind="ExternalInput").ap()
    wq_d = nc.dram_tensor("wq", [128, 4, 2, 512], FP8, kind="ExternalInput").ap()
    wqr_d = nc.dram_tensor("wqr", [128, 4, 2, 512], FP8, kind="ExternalInput").ap()
    wk_d = nc.dram_tensor("wk", [128, 4, 2, 512], FP8, kind="ExternalInput").ap()
    wkr_d = nc.dram_tensor("wkr", [128, 4, 2, 512], FP8, kind="ExternalInput").ap()
    bqt_d = nc.dram_tensor("bqt", [128, 4], F32, kind="ExternalInput").ap()
    bkt_d = nc.dram_tensor("bkt", [128, 4], F32, kind="ExternalInput").ap()
    wv_d = nc.dram_tensor("wv", [128, 4, 2, VC], FP8, kind="ExternalInput").ap()
    wvr_d = nc.dram_tensor("wvr", [128, 4, 2, VC], FP8, kind="ExternalInput").ap()
    bv_d = nc.dram_tensor("bv", [1, VC], F32, kind="ExternalInput").ap()
    wout_d = nc.dram_tensor("wout", [128, 2, E], BF16, kind="ExternalInput").ap()
    tri_d = nc.dram_tensor("tri", [128, 128], BF16, kind="ExternalInput").ap()
    out_d = nc.dram_tensor("out", [S, E], BF16, kind="ExternalOutput").ap()

    with tile.TileContext(nc) as tc, ExitStack() as ctx:
        persist = ctx.enter_context(tc.tile_pool(name="persist", bufs=1))

        hs_sb = persist.tile([128, 4, 2, S], FP8, name="hs", tag="hs")
        hsr_sb = persist.tile([128, 4, 2, S], FP8, name="hsr", tag="hsr")
        wq_sb = persist.tile([128, 4, 2, 512], FP8, name="wq", tag="wq")
        wqr_sb = persist.tile([128, 4, 2, 512], FP8, name="wqr", tag="wqr")
        wk_sb = persist.tile([128, 4, 2, 512], FP8, name="wk", tag="wk")
        wkr_sb = persist.tile([128, 4, 2, 512], FP8, name="wkr", tag="wkr")
        wv_sb = persist.tile([128, 4, 2, VC], FP8, name="wv", tag="wv")
        wvr_sb = persist.tile([128, 4, 2, VC], FP8, name="wvr", tag="wvr")
        wout_sb = persist.tile([128, 2, E], BF16, name="wo", tag="wo")
        tri_sb = persist.tile([128, 128], BF16, name="tri", tag="tri")
        bqt_sb = persist.tile([128, 4], F32, name="bqt", tag="bqt")
        bkt_sb = persist.tile([128, 4], F32, name="bkt", tag="bkt")
        bv_sb = persist.tile([1, VC], F32, name="bv", tag="bv")
        bv_bc = persist.tile([128, VC], F32, name="bvbc", tag="bvbc")

        q_sb = [persist.tile([128, S], BF16, name=f"q{m}", tag=f"q{m}")
                for m in range(4)]
        k_sb = [persist.tile([128, S], BF16, name=f"k{m}", tag=f"k{m}")
                for m in range(4)]
        v_sb = [persist.tile([128, VC], BF16, name=f"v{sc}", tag=f"v{sc}")
                for sc in range(16)]
        attn_sb = [persist.tile([128, S], BF16, name=f"attn{t}", tag=f"attn{t}")
                   for t in range(2)]

        # ---- input DMAs, ordered by first use (12 loads) ----
        dma = nc.sync.dma_start
        dma(out=hs_sb[:, :, :, 0:256], in_=hs_d[:, :, :, 0:256])
        dma(out=wv_sb[:, 0:2, :, :], in_=wv_d[:, 0:2, :, :])
        dma(out=wv_sb[:, 2:4, :, :], in_=wv_d[:, 2:4, :, :])
        dma(out=hsr_sb[:, :, :, 0:256], in_=hsr_d[:, :, :, 0:256])
        dma(out=wvr_sb, in_=wvr_d)
        dma(out=bv_sb, in_=bv_d)
        dma(out=hs_sb[:, :, :, 256:512], in_=hs_d[:, :, :, 256:512])
        dma(out=hsr_sb[:, :, :, 256:512], in_=hsr_d[:, :, :, 256:512])
        dma(out=hs_sb[:, :, :, 512:1024], in_=hs_d[:, :, :, 512:1024])
        dma(out=hsr_sb[:, :, :, 512:1024], in_=hsr_d[:, :, :, 512:1024])
        dma(out=wq_sb, in_=wq_d)
        dma(out=wqr_sb, in_=wqr_d)
        dma(out=bqt_sb, in_=bqt_d)
        dma(out=bkt_sb, in_=bkt_d)
        dma(out=hs_sb[:, :, :, 1024:1536], in_=hs_d[:, :, :, 1024:1536])
        dma(out=hsr_sb[:, :, :, 1024:1536], in_=hsr_d[:, :, :, 1024:1536])
        dma(out=wk_sb, in_=wk_d)
        dma(out=wkr_sb, in_=wkr_d)
        dma(out=hs_sb[:, :, :, 1536:2048], in_=hs_d[:, :, :, 1536:2048])
        dma(out=hsr_sb[:, :, :, 1536:2048], in_=hsr_d[:, :, :, 1536:2048])
        dma(out=tri_sb, in_=tri_d)
        dma(out=wout_sb, in_=wout_d)

        nc.gpsimd.partition_broadcast(bv_bc, bv_sb)

        # ---- helpers ----
        def v_group(pool, sc, bufs):
            # v_aug (64x) = sum over chunk pairs of (hs8+hsr8)^T (wv8+wvr8),
            # dropping the tiny hsr*wvr term; 12 DoubleRow matmuls.
            ps = pool.tile([128, VC], F32, name="vp", tag="vp", bufs=bufs)
            ssl = slice(sc * 128, (sc + 1) * 128)
            terms = [(hs_sb, wv_sb), (hs_sb, wvr_sb), (hsr_sb, wv_sb)]
            n = 0
            for ht, wt in terms:
                for c in range(4):
                    nc.tensor.matmul(ps, ht[:, c, :, ssl], wt[:, c, :, :],
                                     start=(n == 0), stop=(n == 11),
                                     perf_mode=DR)
                    n += 1
            nc.vector.tensor_add(out=v_sb[sc], in0=ps, in1=bv_bc)

        def qk_group(pool, tag, bufs, which, m, nb, copy_engine):
            w_sb, wr_sb, bias_sb, dst = (
                (wq_sb, wqr_sb, bqt_sb, q_sb) if which == "q"
                else (wk_sb, wkr_sb, bkt_sb, k_sb))
            sl = slice(nb * 512, nb * 512 + 512)
            msl = slice(m * 128, (m + 1) * 128)
            ps = pool.tile([128, 512], F32, name="pj", tag=tag, bufs=bufs)
            n = 0
            for ht, wt in ((hs_sb, w_sb), (hs_sb, wr_sb), (hsr_sb, w_sb)):
                for c in range(4):
                    nc.tensor.matmul(ps, wt[:, c, :, msl], ht[:, c, :, sl],
                                     start=(n == 0), stop=(n == 11),
                                     perf_mode=DR)
                    n += 1
            if copy_engine == "act":
                nc.scalar.activation(out=dst[m][:, sl], in_=ps, func=AF.Identity,
                                     bias=bias_sb[:, m:m + 1], scale=1.0)
            else:
                nc.vector.tensor_scalar_add(out=dst[m][:, sl], in0=ps,
                                            scalar1=bias_sb[:, m:m + 1])

        def outproj_pair(pool, tag, bufs, ob_pool, it, copy_engines):
            # both 512-col halves of rows it*128..+128, one merged store
            ot = ob_pool.tile([128, E], BF16, name="ot", tag="ot")
            for eb in range(2):
                sl = slice(eb * 512, eb * 512 + 512)
                ps = pool.tile([128, 512], F32, name="op", tag=tag, bufs=bufs)
                for t in range(2):
                    nc.tensor.matmul(ps, attn_sb[t][:, it * 128:(it + 1) * 128],
                                     wout_sb[:, t, sl],
                                     start=(t == 0), stop=(t == 1))
                if copy_engines[eb] == "act":
                    nc.scalar.activation(out=ot[:, sl], in_=ps, func=AF.Copy,
                                         bias=0.0, scale=1.0)
                else:
                    nc.vector.tensor_copy(out=ot[:, sl], in_=ps)
            nc.sync.dma_start(out=out_d[it * 128:(it + 1) * 128, :], in_=ot)

        # ---- phase 1a: v projection + q/k m=0 cols 0..1024 ----
        with ExitStack() as p1:
            pp0 = p1.enter_context(tc.tile_pool(name="pp0", bufs=1, space="PSUM"))
            for sc in range(12):
                v_group(pp0, sc, bufs=4)
            qk_group(pp0, "pj", 2, "q", 0, 0, "act")
            qk_group(pp0, "pj", 2, "q", 0, 1, "dve")
            qk_group(pp0, "pj", 2, "k", 0, 0, "act")
            qk_group(pp0, "pj", 2, "k", 0, 1, "dve")

        # ---- attention (+ interleaved foreign PE work) ----
        with ExitStack() as actx:
            sp_pool = actx.enter_context(tc.tile_pool(name="spp", bufs=1, space="PSUM"))
            at_pool = actx.enter_context(tc.tile_pool(name="atp", bufs=1, space="PSUM"))
            fp_pool = actx.enter_context(tc.tile_pool(name="fpp", bufs=1, space="PSUM"))
            et_pool = actx.enter_context(tc.tile_pool(name="etp", bufs=4))
            nrm_pool = actx.enter_context(tc.tile_pool(name="nrm", bufs=2))
            ob_pool = actx.enter_context(tc.tile_pool(name="obp", bufs=4))

            # foreign PE work queues; order respects data deps:
            # ib2=0 head h=2m needs q/k m nb0/nb1 emitted by end of head 2m-1;
            # ib2=1 head h uses q cols 1024:2048 (nb2/3) and k chunks jc>=8.
            # qk groups are split into two 4-chunk half-units (same psum tile,
            # accumulation group spans both) so foreign work comes in ~0.85us
            # units that slot between attention chunks at fine grain.
            def qk_units(which, m, nb):
                w_sb, wr_sb, bias_sb, dst = (
                    (wq_sb, wqr_sb, bqt_sb, q_sb) if which == "q"
                    else (wk_sb, wkr_sb, bkt_sb, k_sb))
                sl = slice(nb * 512, nb * 512 + 512)
                msl = slice(m * 128, (m + 1) * 128)
                parts = [(ht, wt, c)
                         for ht, wt in ((hs_sb, w_sb), (hs_sb, wr_sb),
                                        (hsr_sb, w_sb))
                         for c in range(4)]
                state = {}

                def half(lohalf):
                    def emit():
                        if "ps" not in state:
                            state["ps"] = fp_pool.tile([128, 512], F32,
                                                       name="pj", tag="fp",
                                                       bufs=2)
                        ps = state["ps"]
                        rng = range(0, 6) if lohalf else range(6, 12)
                        for n in rng:
                            ht, wt, c = parts[n]
                            nc.tensor.matmul(ps, wt[:, c, :, msl],
                                             ht[:, c, :, sl],
                                             start=(n == 0), stop=(n == 11),
                                             perf_mode=DR)
                        if not lohalf:
                            nc.vector.tensor_scalar_add(
                                out=dst[m][:, sl], in0=ps,
                                scalar1=bias_sb[:, m:m + 1])
                    return emit
                return [half(True), half(False)]

            def op_unit(it):
                def emit():
                    outproj_pair(fp_pool, "fp", 2, ob_pool, it, ("dve", "dve"))
                return emit

            def vp_unit(sc):
                # v projection group as foreign work; borrows an fp-pool slot
                # (tile is [128,512] f32, v psum uses the first 264 cols)
                def emit():
                    ps = fp_pool.tile([128, 512], F32, name="pj", tag="fp",
                                      bufs=2)
                    ssl = slice(sc * 128, (sc + 1) * 128)
                    n = 0
                    for ht, wt in ((hs_sb, wv_sb), (hs_sb, wvr_sb),
                                   (hsr_sb, wv_sb)):
                        for c in range(4):
                            nc.tensor.matmul(ps[:, 0:VC], ht[:, c, :, ssl],
                                             wt[:, c, :, :],
                                             start=(n == 0), stop=(n == 11),
                                             perf_mode=DR)
                            n += 1
                    nc.vector.tensor_add(out=v_sb[sc], in0=ps[:, 0:VC],
                                         in1=bv_bc)
                return emit

            foreignA = []
            for which, m, nb in (("q", 1, 0), ("q", 1, 1), ("k", 1, 0),
                                 ("k", 1, 1), ("q", 2, 0), ("q", 2, 1),
                                 ("k", 2, 0), ("k", 2, 1), ("q", 3, 0),
                                 ("q", 3, 1), ("k", 3, 0), ("k", 3, 1),
                                 ("q", 0, 2), ("q", 0, 3), ("k", 0, 2),
                                 ("k", 0, 3)):
                foreignA += qk_units(which, m, nb)
            foreignB = []
            for which, m, nb in (("q", 1, 2), ("q", 1, 3), ("k", 1, 2),
                                 ("k", 1, 3), ("q", 2, 2), ("q", 2, 3),
                                 ("k", 2, 2), ("k", 2, 3), ("q", 3, 2),
                                 ("q", 3, 3), ("k", 3, 2), ("k", 3, 3)):
                foreignB += qk_units(which, m, nb)
            # order check (deps): q m nb2+nb3 complete before head 2m of
            # ib2=1 starts; k m nb2 before that head's jc=8, nb3 before jc=12

            foreignB += [op_unit(it) for it in range(8)]
            foreignB = [vp_unit(sc) for sc in range(12, 16)] + foreignB
            fq = {0: foreignA, 1: foreignB}

            # pop budgets tuned to the per-head ACT deficit: late ib2=1 heads
            # (where proj work has run out) get the outproj units
            def points_for(ib2, h):
                if ib2 == 0:
                    return (2, 4, 6)          # + boundary = 4 units/head
                if h == 0:
                    return (1, 2, 3, 5, 7)    # + boundary = 6 (vp12..15 early)
                if h < 6:
                    return (4, 8, 12)         # + boundary = 4 units/head
                return (3, 6, 9, 12)          # + boundary = 5 units/head

            def pop_foreign(ib2):
                q = fq[ib2]
                if q:
                    q.pop(0)()

            for ib2 in range(2):
                ibase = ib2 * 1024
                jcs = list(range(8 * (ib2 + 1)))
                ilo = {jc: max(jc * 128 - ibase, 0) for jc in jcs}
                bank_jcs = {nb: [jc for jc in jcs if ilo[jc] < nb * 512 + 512]
                            for nb in range(2)}

                for h in range(8):
                    pairm = h // 2
                    dpart = slice((h % 2) * 64, (h % 2) * 64 + 64)
                    at_ps = at_pool.tile([33, 1024], F32, name="at", tag="at",
                                         bufs=1)
                    ets = {}

                    def emit_qk_exp(jc):
                        lo = ilo[jc]
                        sp = sp_pool.tile([128, 1024], F32, name="sp", tag="sp",
                                          bufs=2)
                        for nb in range(2):
                            a = max(lo, nb * 512)
                            bb = nb * 512 + 512
                            if a >= bb:
                                continue
                            nc.tensor.matmul(
                                sp[:, a:bb],
                                k_sb[pairm][dpart, jc * 128:(jc + 1) * 128],
                                q_sb[pairm][dpart, ibase + a:ibase + bb],
                                start=True, stop=True)
                        et = et_pool.tile([128, 1024], BF16, name="et", tag="et",
                                          bufs=4)
                        nc.scalar.activation(out=et[:, lo:1024], in_=sp[:, lo:1024],
                                             func=AF.Exp, scale=SCALE)
                        if jc >= 8 * ib2:  # diagonal block: causal mask
                            nc.vector.tensor_mul(out=et[:, lo:lo + 128],
                                                 in0=et[:, lo:lo + 128],
                                                 in1=tri_sb)
                        ets[jc] = et

                    def emit_pv(jc):
                        lo = ilo[jc]
                        et = ets.pop(jc)
                        for nb in range(2):
                            a = max(lo, nb * 512)
                            bb = nb * 512 + 512
                            if a >= bb:
                                continue
                            nc.tensor.matmul(
                                at_ps[:, a:bb],
                                v_sb[jc][:, h * 33:(h + 1) * 33],
                                et[:, a:bb],
                                start=(jc == bank_jcs[nb][0]),
                                stop=(jc == bank_jcs[nb][-1]))

                    pts = points_for(ib2, h)
                    # per-bank normalize: bank nb's accumulation stops at
                    # bank_jcs[nb][-1], so its staging copy + recip + bcast +
                    # mul can run while later chunks still accumulate bank 1.
                    # This frees the at-psum bank for the next head's PV and,
                    # on the last head, unlocks the tail outproj ~4 chunks
                    # earlier.
                    t, roff = h // 4, (h % 4) * 32
                    araw = nrm_pool.tile([33, 1024], BF16, name="araw", tag="araw")
                    rec = nrm_pool.tile([1, 1024], BF16, name="rec", tag="rec")
                    rec_bc = nrm_pool.tile([32, 1024], BF16, name="recbc",
                                           tag="recbc")

                    def norm_bank(nb):
                        ca, cb = nb * 512, nb * 512 + 512
                        nc.vector.tensor_copy(out=araw[:, ca:cb],
                                              in_=at_ps[:, ca:cb])
                        with nc.allow_low_precision(reason="bf16 softmax denom"):
                            nc.vector.reciprocal(out=rec[:, ca:cb],
                                                 in_=araw[32:33, ca:cb])
                        nc.gpsimd.partition_broadcast(rec_bc[:, ca:cb],
                                                      rec[:, ca:cb])
                        nc.vector.tensor_mul(
                            out=attn_sb[t][roff:roff + 32, ibase + ca:ibase + cb],
                            in0=araw[0:32, ca:cb], in1=rec_bc[:, ca:cb])

                    for idx, jc in enumerate(jcs):
                        emit_qk_exp(jc)
                        if idx >= 1:
                            emit_pv(jcs[idx - 1])
                            if jcs[idx - 1] == bank_jcs[0][-1]:
                                norm_bank(0)
                        if idx in pts:
                            pop_foreign(ib2)
                    emit_pv(jcs[-1])
                    if jcs[-1] == bank_jcs[0][-1]:
                        norm_bank(0)
                    pop_foreign(ib2)
                    norm_bank(1)

            while fq[1]:
                pop_foreign(1)


        # ---- tail: outproj rows 1024..2047 ----
        with ExitStack() as tctx:
            tp = tctx.enter_context(tc.tile_pool(name="tp", bufs=1, space="PSUM"))
            obt = tctx.enter_context(tc.tile_pool(name="obt", bufs=4))
            for it in range(8, 16):
                outproj_pair(tp, "tp", 6, obt, it, ("act", "dve"))

    nc.compile()
    return nc


def _get_program():
    if "nc" not in _PROGRAM_CACHE:
        _PROGRAM_CACHE["nc"] = _build_program()
    return _PROGRAM_CACHE["nc"]


E4M3 = ml_dtypes.float8_e4m3  # what the stack maps dt.float8e4 to


def _fp8_pair(a):
    """[E, N] f32 -> (value, residual) in chunk-pair plane layout
    [128, 4, 2, N] fp8: plane p of pair c holds rows (2c+p)*128..+128."""
    a8 = a.astype(E4M3)
    r8 = (a - a8.astype(np.float32)).astype(E4M3)
    out = []
    for x in (a8, r8):
        x = x.reshape(4, 2, 128, *a.shape[1:])
        x = np.ascontiguousarray(x.transpose(2, 0, 1, 3))
        out.append(x)
    return out


def kernel(hidden_states, q_weight, q_bias, k_weight, k_bias,
           low_rank_value_weight, low_rank_value_bias,
           low_rank_output_weight, output_bias):
    hidden_states = np.asarray(hidden_states, dtype=np.float32)
    q_weight = np.asarray(q_weight, dtype=np.float32)
    q_bias = np.asarray(q_bias, dtype=np.float32)
    k_weight = np.asarray(k_weight, dtype=np.float32)
    k_bias = np.asarray(k_bias, dtype=np.float32)
    wv_full = np.asarray(low_rank_value_weight, dtype=np.float32)
    bv_full = np.asarray(low_rank_value_bias, dtype=np.float32)
    wout_full = np.asarray(low_rank_output_weight, dtype=np.float32)
    output_bias = np.asarray(output_bias, dtype=np.float32)

    tri = np.triu(np.ones((128, 128), np.float32)).astype(ml_dtypes.bfloat16)
    ws = np.float32(WS)

    in_maps = []
    for c in range(N_CORES):
        b, g = c // 2, c % 2
        hs_t = np.ascontiguousarray(hidden_states[b].T)          # [E, S]
        cols = slice(g * 512, (g + 1) * 512)                     # q/k head cols
        vcols = slice(g * 256, (g + 1) * 256)                    # v head cols
        wv_aug = np.zeros((E, VC), dtype=np.float32)
        bv_aug = np.zeros((1, VC), dtype=np.float32)
        wv_g = wv_full[:, vcols].reshape(E, HG, R)
        bv_g = bv_full[vcols].reshape(HG, R)
        for h in range(HG):
            wv_aug[:, h * 33:h * 33 + 32] = wv_g[:, h, :]
            bv_aug[0, h * 33:h * 33 + 32] = bv_g[h]
            bv_aug[0, h * 33 + 32] = 1.0
        hs8, hsr8 = _fp8_pair(hs_t)
        wq8, wqr8 = _fp8_pair(ws * q_weight[:, cols])
        wk8, wkr8 = _fp8_pair(ws * k_weight[:, cols])
        wv8, wvr8 = _fp8_pair(ws * wv_aug)
        wout_r = np.ascontiguousarray(
            wout_full[vcols, :].reshape(2, 128, E).transpose(1, 0, 2)
        ).astype(ml_dtypes.bfloat16)
        in_maps.append({
            "hs": hs8, "hsr": hsr8,
            "wq": wq8, "wqr": wqr8,
            "wk": wk8, "wkr": wkr8,
            "bqt": np.ascontiguousarray(ws * q_bias[cols].reshape(4, 128).T),
            "bkt": np.ascontiguousarray(ws * k_bias[cols].reshape(4, 128).T),
            "wv": wv8, "wvr": wvr8,
            "bv": ws * bv_aug,
            "wout": wout_r,
            "tri": tri,
        })

    nc = _get_program()
    res = run_bass_kernel_spmd(nc, in_maps, list(range(N_CORES)))
    out = np.empty((B, S, E), dtype=np.float32)
    for b in range(B):
        out[b] = (res.results[2 * b]["out"].astype(np.float32)
                  + res.results[2 * b + 1]["out"].astype(np.float32)
                  + output_bias[None, :])
    return out
